# revision 23
# baseline (speedup 1.0000x reference)
"""TRN2 Bass kernel for nn_CrossAttention (sparse channel attention + prompt
fusion), sharded spatially over 8 NeuronCores.  Self-contained: builds the
SPMD Bass/Tile program once, shards the full inputs host-side (16 image rows
per core + halo), runs via run_bass_kernel_spmd, and reassembles the output.

Pipelined structure: per-batch Gram AllReduces overlap the other batch's
convolution work and the first batch's attention tail:
  A0 -> AR0 || A1 -> AR1 || B0,C0 -> B1,C1
"""
import sys

for _p in ("/opt/trn_rl_repo", "/root/.axon_site/_ro/trn_rl_repo"):
    if _p not in sys.path:
        sys.path.insert(0, _p)

import numpy as np

B, DIM, HEADS, Himg, Wimg = 2, 384, 8, 128, 128
C = DIM // HEADS            # 48
QKVC = 3 * DIM              # 1152
NCORE = 8
ROWS = Himg // NCORE        # 16 rows per core
NL = ROWS * Wimg            # 2048 local pixels
HR = ROWS + 2               # 18 rows with halo
NH = HR * Wimg              # 2304 halo pixels
PADW = Wimg + 2             # 130
NPAD = HR * PADW            # 2340 padded-free size
NSEG = 3                    # 384 attention rows per batch / 128
NBH = HEADS                 # 8 (per batch)


def build_dw_units():
    """Returns (units, perm) where units is a list of dicts and perm maps
    raw qkv channel -> (in_tile m, in_part p).  44 units per batch.

    unit: raw_base, length, in_tile, in_block(i), in_off, out_kind('q'/'k'/'v'),
          out_tile, out_base (partition base of its out run), j (out col_grp)
    """
    units = []
    for kind, koff in (("q", 0), ("k", DIM)):
        for h in range(HEADS):
            units.append(dict(kind=kind, raw_base=koff + C * h, length=32,
                              out_tile=h // 2, out_base=64 * (h % 2),
                              j=2 * (h % 2), half=False))
            units.append(dict(kind=kind, raw_base=koff + C * h + 32, length=16,
                              out_tile=h // 2, out_base=64 * (h % 2) + 32,
                              j=2 * (h % 2) + 1, half=True))
    for t in range(12):
        units.append(dict(kind="v", raw_base=2 * DIM + 32 * t, length=32,
                          out_tile=t // 4, out_base=32 * (t % 4),
                          j=t % 4, half=False))

    # Three execution sub-passes, each confined to 3 input tiles so only
    # [128, 3*NPAD] of qkv-pad SBUF is live at a time:
    #   sp0: q/k heads 0-3 -> in_tiles 0-2; sp1: q/k heads 4-7 -> 3-5;
    #   sp2: v -> 6-8.
    # Within each sub-pass, each j class has exactly 4 units -> assign
    # distinct i (round robin), giving (i, j) sub-array depth 1 per pass.
    def subpass_of(u):
        if u["kind"] == "v":
            return 2
        return 0 if (u["raw_base"] % DIM) < 4 * C else 1

    jc_ctr = {}
    for u in units:
        key = (subpass_of(u), u["j"])
        o = jc_ctr.get(key, 0)
        jc_ctr[key] = o + 1
        # stagger v's 3-members-per-j classes so all four i values are used
        u["i"] = (u["j"] + o) % 4 if u["kind"] == "v" else o
    assert all(u["i"] < 4 for u in units)

    # pack into input slots within each sub-pass's 3 tiles
    slot_next = {}
    half_open = {}
    for u in units:
        spi, i = subpass_of(u), u["i"]
        key = (spi, i)
        if not u["half"]:
            m = 3 * spi + slot_next.get(key, 0)
            slot_next[key] = slot_next.get(key, 0) + 1
            u["in_tile"], u["in_off"] = m, 0
        else:
            if key in half_open:
                u["in_tile"], u["in_off"] = half_open.pop(key), 16
            else:
                m = 3 * spi + slot_next.get(key, 0)
                slot_next[key] = slot_next.get(key, 0) + 1
                half_open[key] = m
                u["in_tile"], u["in_off"] = m, 0
    assert not half_open, half_open
    assert all(v <= 3 for v in slot_next.values()), slot_next
    for u in units:
        assert 3 * subpass_of(u) <= u["in_tile"] < 3 * subpass_of(u) + 3

    # column base in the dwdiag weight table (per-i column space is shared
    # across the 4 partition groups)
    slot_ctr = {i: 0 for i in range(4)}
    for u in units:
        i = u["i"]
        u["colbase"] = slot_ctr[i] * 9 * 32
        slot_ctr[i] += 1

    # permutation: raw qkv channel -> (m, p)
    perm = np.full((QKVC, 2), -1, np.int64)
    for u in units:
        for r in range(u["length"]):
            raw = u["raw_base"] + r
            p = 32 * u["i"] + u["in_off"] + r
            perm[raw] = (u["in_tile"], p)
    assert (perm >= 0).all()
    return units, perm


def prep_constants(inputs):
    """Build all host-side DRAM input arrays (weights, tables) shared by all cores."""
    import ml_dtypes
    bf16 = ml_dtypes.bfloat16
    units, perm = build_dw_units()
    w_qkv = np.asarray(inputs["w_qkv"], np.float32)      # [1152, 384]
    w_dw = np.asarray(inputs["w_dw"], np.float32).reshape(QKVC, 9)
    w_proj = np.asarray(inputs["w_proj"], np.float32)    # [384, 384]

    # wqkvT: [3 kt, 128, 1152] with cols in PERMUTED order (m*128+p)
    wqkvT = np.zeros((3, 128, QKVC), np.float32)
    for raw in range(QKVC):
        m, p = perm[raw]
        wqkvT[:, :, m * 128 + p] = w_qkv[raw].reshape(3, 128)

    # dwdiag: [128, COLS] bf16; per unit u, tap t a [32,32] block at
    # partitions 32*i..+32, columns colbase(u,t)..+32.
    maxu_per_i = max(sum(1 for u in units if u["i"] == i) for i in range(4))
    COLS = maxu_per_i * 9 * 32
    dwdiag = np.zeros((128, COLS), np.float32)
    for u in units:
        i = u["i"]
        for t in range(9):
            cb = u["colbase"] + t * 32
            for c in range(u["length"]):
                raw = u["raw_base"] + c
                r = u["in_off"] + c
                dwdiag[32 * i + r, cb + c] = w_dw[raw, t]

    def lhsT3(w):   # w [out, in=384] -> [3, 128, out]
        return np.transpose(np.asarray(w, np.float32).reshape(-1, 3, 128), (1, 2, 0)).copy()

    # sel3: [8, 3*128] f32: sel3[j, s*128+p] = 1 if (128s+p)//48 == j
    sel = np.zeros((NBH, NSEG * 128), np.float32)
    for s in range(NSEG):
        for p in range(128):
            rr = 128 * s + p
            sel[rr // 48, s * 128 + p] = 1.0

    # w2rep: [3, 128, 64] — w2's k-tile broadcast across 64 columns, so the
    # gate matmul produces the per-pixel gate replicated over 64 partitions.
    def w2rep(w):
        r = np.asarray(w, np.float32).reshape(3, 128, 1)
        return np.repeat(r, 64, axis=2)

    # wtT_pack: [128, 384] — chr on partitions 0:64, detg on 64:128, for
    # row-packed concurrent branch matmuls.
    wtT_pack = np.concatenate(
        [np.asarray(inputs["chr_wt"], np.float32).T,
         np.asarray(inputs["detg_wt"], np.float32).T], axis=0)

    out = dict(
        wqkvT=wqkvT.astype(bf16),
        dwdiag=dwdiag.astype(bf16),
        wprojT=lhsT3(w_proj).astype(bf16),
        w1T_chr=lhsT3(inputs["chr_w1"]).astype(bf16),
        w1T_detg=lhsT3(inputs["detg_w1"]).astype(bf16),
        w2rep_chr=w2rep(inputs["chr_w2"]).astype(bf16),
        w2rep_detg=w2rep(inputs["detg_w2"]).astype(bf16),
        wtT_pack=wtT_pack.astype(bf16),
        b2col=np.tile(np.asarray([float(np.asarray(inputs["chr_b2"]).ravel()[0]),
                                  float(np.asarray(inputs["detg_b2"]).ravel()[0])],
                                 np.float32), (128, 1)),
        b1_chr=np.ascontiguousarray(np.asarray(inputs["chr_b1"], np.float32).reshape(3, 128).T),
        b1_detg=np.ascontiguousarray(np.asarray(inputs["detg_b1"], np.float32).reshape(3, 128).T),
        bt_chr=np.ascontiguousarray(np.asarray(inputs["chr_bt"], np.float32).reshape(3, 128).T),
        bt_detg=np.ascontiguousarray(np.asarray(inputs["detg_bt"], np.float32).reshape(3, 128).T),
        temp_rep=np.asarray(inputs["temperature"], np.float32).reshape(NBH, 1).copy(),
        attns2=np.asarray(inputs["attns"], np.float32).reshape(1, 2).copy(),
        detg_z=np.asarray(inputs["detg_z"], np.float32).reshape(1, 64).copy(),
        sel3=sel,
    )
    return out, units, perm


def shard_inputs(inputs, consts):
    """Per-core input maps: x slices (bf16, halo-padded), gk slices (bf16)."""
    import ml_dtypes
    bf16 = ml_dtypes.bfloat16
    x = np.asarray(inputs["x"], np.float32)      # [B, 384, 128, 128]
    gk0 = np.asarray(inputs["gk0"], np.float32)  # [B, 64, 128, 128]
    gk1 = np.asarray(inputs["gk1"], np.float32)
    xp = np.pad(x, ((0, 0), (0, 0), (1, 1), (0, 0)))   # zero halo rows
    maps = []
    for ci in range(NCORE):
        r0 = ROWS * ci
        xs = xp[:, :, r0:r0 + HR, :]                       # [B, 384, 18, 128]
        xs = xs.reshape(B, 3, 128, NH)                     # channel-tiled
        g0 = gk0[:, :, r0:r0 + ROWS, :].reshape(B, 64, NL)
        g1 = gk1[:, :, r0:r0 + ROWS, :].reshape(B, 64, NL)
        m = {"x_s": np.ascontiguousarray(xs).astype(bf16),
             "gk0_s": np.ascontiguousarray(g0).astype(bf16),
             "gk1_s": np.ascontiguousarray(g1).astype(bf16)}
        m.update({k: v for k, v in consts.items()})
        maps.append(m)
    return maps


from contextlib import ExitStack


import concourse.bass as bass
import concourse.tile as tile
import concourse.mybir as mybir
from concourse import bacc
from concourse.masks import make_identity

f32 = mybir.dt.float32
bf16 = mybir.dt.bfloat16
AX = mybir.AxisListType
OP = mybir.AluOpType
AF = mybir.ActivationFunctionType
CH = 512          # pixel chunk for most matmuls
NCHUNK = NL // CH  # 4


def build_program():
    units, _ = build_dw_units()
    maxu = max(sum(1 for u in units if u["i"] == i) for i in range(4))
    DWCOLS = maxu * 9 * 32

    nc = bacc.Bacc("TRN2", debug=False, num_devices=NCORE,
                   target_bir_lowering=False)

    def din(name, shape, dt=bf16):
        return nc.dram_tensor(name, list(shape), dt, kind="ExternalInput").ap()

    x_s = din("x_s", (B, 3, 128, NH))
    gk0_s = din("gk0_s", (B, 64, NL))
    gk1_s = din("gk1_s", (B, 64, NL))
    wqkvT_d = din("wqkvT", (3, 128, QKVC))
    dwdiag_d = din("dwdiag", (128, DWCOLS))
    wprojT_d = din("wprojT", (3, 128, DIM))
    w1T_d = {"chr": din("w1T_chr", (3, 128, DIM)), "detg": din("w1T_detg", (3, 128, DIM))}
    w2rep_d = {"chr": din("w2rep_chr", (3, 128, 64)), "detg": din("w2rep_detg", (3, 128, 64))}
    wtTp_d = din("wtT_pack", (128, DIM))
    b1_d = {"chr": din("b1_chr", (128, 3), f32), "detg": din("b1_detg", (128, 3), f32)}
    bt_d = {"chr": din("bt_chr", (128, 3), f32), "detg": din("bt_detg", (128, 3), f32)}
    b2col_d = din("b2col", (128, 2), f32)
    temp_d = din("temp_rep", (NBH, 1), f32)
    attns_d = din("attns2", (1, 2), f32)
    zrow_d = din("detg_z", (1, 64), f32)
    sel3_d = din("sel3", (NBH, NSEG * 128), f32)

    OUT = nc.dram_tensor("OUT", [B, 3, 128, NL], bf16, kind="ExternalOutput").ap()

    # internal DRAM (per batch)
    g3_part = [nc.dram_tensor(f"g3_part{b}", [NBH, 96, 96], f32) for b in range(B)]
    # compact collective buffer: qk cross blocks [(h c) d] then norms [h, 96]
    NQK = NBH * 48 * 48
    g3c_part = [nc.dram_tensor(f"g3c_part{b}", [NQK + NBH * 96], f32) for b in range(B)]
    g3c_all = [nc.dram_tensor(f"g3c_all{b}", [NQK + NBH * 96], f32, addr_space="Shared")
               for b in range(B)]
    rq_flat = [nc.dram_tensor(f"rq_flat{b}", [NSEG * 128], f32) for b in range(B)]
    ac_flat = [nc.dram_tensor(f"ac_flat{b}", [NSEG * 128 * 48], f32) for b in range(B)]
    acT_dram = [nc.dram_tensor(f"acT_dram{b}", [NBH * 48 * 48], bf16) for b in range(B)]
    zb_d = nc.dram_tensor("zb", [64], f32)

    with tile.TileContext(nc) as tc, ExitStack() as ctx:
        _body(tc, ctx, units, locals())
    nc.compile()
    return nc


def _body(tc, ctx, units, t):
    nc = tc.nc
    ec = [0]

    def ecopy(out_ap, in_ap):
        if ec[0] % 2 == 0:
            nc.scalar.copy(out_ap, in_ap)
        else:
            nc.vector.tensor_copy(out_ap, in_ap)
        ec[0] += 1

    wp = ctx.enter_context(tc.tile_pool(name="wp", bufs=1))
    pp = ctx.enter_context(tc.tile_pool(name="pp", bufs=1))     # phase-A big
    cp = ctx.enter_context(tc.tile_pool(name="cp", bufs=1))     # phase-C big
    sp = ctx.enter_context(tc.tile_pool(name="sp", bufs=1))     # small scratch
    ps_pool = ctx.enter_context(tc.tile_pool(name="ps", bufs=2, space="PSUM"))

    # ---------------- constants into SBUF ----------------
    wqkvT = [wp.tile([128, QKVC], bf16, tag=f"wqkv{k}", name=f"wqkv{k}") for k in range(3)]
    for k in range(3):
        nc.sync.dma_start(wqkvT[k][:], t["wqkvT_d"][k])
    dwdiag = wp.tile([128, t["dwdiag_d"].shape[1]], bf16, tag="dwdiag", name="dwdiag")
    nc.sync.dma_start(dwdiag[:], t["dwdiag_d"][:])
    wprojT = [wp.tile([128, DIM], bf16, tag=f"wproj{k}", name=f"wproj{k}") for k in range(3)]
    for k in range(3):
        nc.sync.dma_start(wprojT[k][:], t["wprojT_d"][k])
    w1T, w2r, b1, bt = {}, {}, {}, {}
    for br in ("chr", "detg"):
        w1T[br] = [wp.tile([128, DIM], bf16, tag=f"w1{br}{k}", name=f"w1{br}{k}") for k in range(3)]
        for k in range(3):
            nc.sync.dma_start(w1T[br][k][:], t["w1T_d"][br][k])
        w2r[br] = [wp.tile([128, 64], bf16, tag=f"w2{br}{k}", name=f"w2{br}{k}") for k in range(3)]
        for k in range(3):
            nc.sync.dma_start(w2r[br][k][:], t["w2rep_d"][br][k])
        b1[br] = wp.tile([128, 3], f32, tag=f"b1{br}", name=f"b1{br}")
        nc.sync.dma_start(b1[br][:], t["b1_d"][br][:])
        bt[br] = wp.tile([128, 3], f32, tag=f"bt{br}", name=f"bt{br}")
        nc.sync.dma_start(bt[br][:], t["bt_d"][br][:])
    wtTp = wp.tile([128, DIM], bf16, tag="wtTp", name="wtTp")
    nc.sync.dma_start(wtTp[:], t["wtTp_d"][:])
    b2col = wp.tile([128, 2], f32, tag="b2col", name="b2col")
    nc.sync.dma_start(b2col[:], t["b2col_d"][:])
    tempc = wp.tile([NBH, 1], f32, tag="temp", name="temp")
    nc.sync.dma_start(tempc[:], t["temp_d"][:])
    attns_sb = wp.tile([1, 2], f32, tag="attns", name="attns")
    nc.sync.dma_start(attns_sb[:], t["attns_d"][:])
    zrow = wp.tile([1, 64], f32, tag="zrow", name="zrow")
    nc.sync.dma_start(zrow[:], t["zrow_d"][:])
    sel3 = wp.tile([NBH, NSEG * 128], f32, tag="sel3", name="sel3")
    nc.sync.dma_start(sel3[:], t["sel3_d"][:])
    ident = wp.tile([64, 64], f32, tag="ident", name="ident")
    make_identity(nc, ident[:])
    ones1f = wp.tile([1, 128], f32, tag="ones1f", name="ones1f")
    nc.vector.memset(ones1f[:], 1.0)

    # attns broadcast to all partitions via fp32 K=1 matmul
    ps_a = ps_pool.tile([128, 2], f32, tag="dw0", name="attnsps")
    nc.tensor.matmul(ps_a[:], ones1f[:], attns_sb[:], start=True, stop=True)
    attns_bc = wp.tile([128, 2], f32, tag="attnsbc", name="attnsbc")
    nc.vector.tensor_copy(attns_bc[:], ps_a[:])

    # z-bar prep: z / max(||z||, 1e-12), replicated on partitions 64:128
    # (the alpha matmul's rhs gk-detg lives at partitions 64:128)
    zsq = sp.tile([1, 64], f32, tag="zsq", name="zsq")
    nc.scalar.square(zsq[:], zrow[:])
    zss = sp.tile([1, 1], f32, tag="zss", name="zss")
    nc.vector.reduce_sum(zss[:], zsq[:], axis=AX.X)
    nc.scalar.sqrt(zss[:], zss[:])
    nc.vector.tensor_scalar_max(zss[:], zss[:], 1e-12)
    zrs = sp.tile([1, 1], f32, tag="zrs", name="zrs")
    nc.vector.reciprocal(zrs[:], zss[:])
    zn = sp.tile([1, 64], f32, tag="zn", name="zn")
    nc.vector.tensor_scalar_mul(zn[:], zrow[:], zrs[:, 0:1])
    nc.sync.dma_start(t["zb_d"].ap().rearrange("(a b) -> a b", a=1), zn[:])
    zcol = sp.tile([128, 1], f32, tag="zcol", name="zcol")
    nc.sync.dma_start(zcol[64:128, :], t["zb_d"].ap().rearrange("(p a) -> p a", a=1))
    ones128f = wp.tile([128, 128], f32, tag="ones128f", name="ones128f")
    nc.vector.memset(ones128f[:], 1.0)
    zrep = sp.tile([128, 128], f32, tag="zrep", name="zrep")
    nc.vector.tensor_scalar_mul(zrep[64:128, :], ones128f[64:128, :], zcol[64:128, 0:1])
    zrep16 = wp.tile([128, 128], bf16, tag="zrep16", name="zrep16")
    nc.vector.tensor_copy(zrep16[64:128, :], zrep[64:128, :])

    # ---------------- per-batch tensors ----------------
    vcm = [[pp.tile([128, NL], bf16, tag=f"v{b}_{mv}", name=f"v{b}_{mv}") for mv in range(3)]
           for b in range(B)]

    units_by_sp = [
        [u for u in units if u["kind"] in "qk" and u["raw_base"] % DIM < 4 * C],
        [u for u in units if u["kind"] in "qk" and u["raw_base"] % DIM >= 4 * C],
        [u for u in units if u["kind"] == "v"],
    ]

    # =============== phase A: qkv conv + dw conv + Gram ===============
    def conv_subpass(b, spi, x_sb, qpad, kpad):
        us = units_by_sp[spi]
        # qkv conv for this group's 3 input tiles (perm channels 3*spi..)
        qkvpad = pp.tile([128, 3 * NPAD], bf16, tag="bigA", name=f"qkvpad_{b}")
        for mg in range(3):
            m = 3 * spi + mg
            pv = qkvpad[:, mg * NPAD:(mg + 1) * NPAD].rearrange(
                "p (r w) -> p r w", w=PADW)
            nc.vector.memset(pv[:, :, 0:1], 0.0)
            nc.vector.memset(pv[:, :, PADW - 1:PADW], 0.0)
            for nck in range(6):           # 6 x 384-pixel chunks (3 rows)
                psq = ps_pool.tile([128, 384], f32, tag=f"dw{nck % 4}", name="qkvps")
                for k in range(3):
                    nc.tensor.matmul(
                        psq[:], wqkvT[k][:, m * 128:(m + 1) * 128],
                        x_sb[k][:, nck * 384:(nck + 1) * 384],
                        start=(k == 0), stop=(k == 2))
                ecopy(pv[:, 3 * nck:3 * nck + 3, 1:129],
                      psq[:].rearrange("p (r w) -> p r w", w=128))

        # dw conv sub-pass
        outkeys = sorted({(u["kind"], u["out_tile"]) for u in us})
        for ck in range(NCHUNK):
            pso = {ok: ps_pool.tile([128, CH], f32, tag=f"dw{oi}", name=f"dw{ok[0]}{ok[1]}")
                   for oi, ok in enumerate(outkeys)}
            for tap in range(9):
                dy, dx = tap // 3, tap % 3
                for u in us:
                    mg = u["in_tile"] - 3 * spi
                    src = qkvpad[32 * u["i"]:32 * u["i"] + 32,
                                 mg * NPAD:(mg + 1) * NPAD]
                    rhs = src.rearrange("p (r w) -> p r w", w=PADW)[
                        :, 4 * ck + dy: 4 * ck + dy + 4, dx:dx + 128]
                    lhsT = dwdiag[32 * u["i"]:32 * u["i"] + 32,
                                  u["colbase"] + tap * 32: u["colbase"] + tap * 32 + 32]
                    ob = u["out_base"]
                    out = pso[(u["kind"], u["out_tile"])][ob:ob + 32, :]
                    nc.tensor.matmul(out, lhsT, rhs,
                                     start=(tap == 0), stop=(tap == 8),
                                     tile_position=(32 * u["i"], ob),
                                     skip_group_check=True)
            for (kind, ot), ps in pso.items():
                dst = {"q": qpad, "k": kpad, "v": vcm[b]}[kind][ot]
                if kind == "v":
                    ecopy(dst[:, ck * CH:(ck + 1) * CH], ps[:])
                else:
                    # one copy covers both 48-runs (partitions 0:48 and
                    # 64:112); 48:64 carries junk that nothing reads
                    ecopy(dst[0:112, ck * CH:(ck + 1) * CH], ps[0:112, :])

    def gram_half(b, half, qpad, kpad, g3sb):
        s_pm = pp.tile([128, 16 * 384], bf16, tag="spm", name=f"s_pm_{b}_{half}")
        spm3 = s_pm[:].rearrange("p (c blk) -> p c blk", blk=384)
        for hh in range(4):
            h = 4 * half + hh
            for qk, koff in ((qpad, 0), (kpad, 48)):
                src = qk[h // 2][64 * (h % 2):64 * (h % 2) + 48, :]
                nc.sync.dma_start_transpose(
                    spm3[:, :, 96 * hh + koff: 96 * hh + koff + 48], src)
        for hh in range(4):
            h = 4 * half + hh
            # 128-wide stationary triggers the compiler's fast-weight-load;
            # rows 96:128 of the result are junk (next head's columns).
            # hh=3 has no 128-wide window inside the half, so it stays 96.
            wn = 128 if hh < 3 else 96
            psg = ps_pool.tile([128, 96], f32, tag=f"dw{hh % 4}", name="g3ps")
            for ckk in range(16):
                nc.tensor.matmul(psg[0:wn, :], spm3[:, ckk, 96 * hh:96 * hh + wn],
                                 spm3[:, ckk, 96 * hh:96 * hh + 96],
                                 start=(ckk == 0), stop=(ckk == 15))
            nc.vector.tensor_copy(g3sb[:, 96 * h:96 * (h + 1)],
                                  psg[0:96, :])

    def phase_A(b):
        x_sb = [pp.tile([128, NH], bf16, tag=f"x{k}", name=f"x{k}_{b}") for k in range(3)]
        for k in range(3):
            nc.sync.dma_start(x_sb[k][:], t["x_s"][b, k])
        qpad = [pp.tile([128, NL], bf16, tag=f"qk_{i2}", name=f"qpad{i2}_{b}") for i2 in range(4)]
        kpad = [pp.tile([128, NL], bf16, tag=f"qk_{4 + i2}", name=f"kpad{i2}_{b}") for i2 in range(4)]
        g3sb = pp.tile([96, 8 * 96], f32, tag="g3sb", name=f"g3sb_{b}")

        # q/k sub-passes first, each followed by its Gram half, so the
        # AllReduce can start before the v sub-pass runs.
        for spi in (0, 1):
            conv_subpass(b, spi, x_sb, qpad, kpad)
            gram_half(b, spi, qpad, kpad, g3sb)
        nc.sync.dma_start(
            t["g3_part"][b].ap().rearrange("h r c -> r h c"),
            g3sb[:].rearrange("r (h c) -> r h c", c=96))
        # compact extraction (DRAM->DRAM): qk cross block + the two diagonals
        gp = t["g3_part"][b]
        gc = t["g3c_part"][b]
        NQK = NBH * 48 * 48
        nc.sync.dma_start(
            gc.ap()[0:NQK].rearrange("(h c d) -> h c d", h=NBH, c=48),
            gp.ap()[:, 0:48, 48:96])
        with nc.allow_non_contiguous_dma(reason="96-element diag extraction"):
            nc.sync.dma_start(
                gc.ap()[NQK:NQK + NBH * 96].rearrange("(h c) -> h c", h=NBH)[:, 0:48],
                bass.AP(tensor=gp, offset=0, ap=[[96 * 96, NBH], [97, 48]]))
            nc.sync.dma_start(
                gc.ap()[NQK:NQK + NBH * 96].rearrange("(h c) -> h c", h=NBH)[:, 48:96],
                bass.AP(tensor=gp, offset=48 * 96 + 48, ap=[[96 * 96, NBH], [97, 48]]))
        conv_subpass(b, 2, x_sb, qpad, kpad)

    # =============== phase B: attention matrices (per batch) ===============
    def phase_B(b):
        # norms arrive compact: [8, 96] (qq diag | kk diag)
        norm2 = sp.tile([NBH, 96], f32, tag="norm2", name=f"norm2_{b}")
        NQK = NBH * 48 * 48
        nc.sync.dma_start(
            norm2[:],
            t["g3c_all"][b].ap()[NQK:NQK + NBH * 96].rearrange("(h c) -> h c", h=NBH))
        nc.scalar.sqrt(norm2[:], norm2[:])
        nc.vector.tensor_scalar_max(norm2[:], norm2[:], 1e-12)
        rn = sp.tile([NBH, 96], f32, tag="rn", name=f"rn_{b}")
        nc.vector.reciprocal(rn[:], norm2[:])
        rqf = sp.tile([NBH, 48], f32, tag="rqf", name=f"rqf_{b}")
        nc.vector.tensor_scalar_mul(rqf[:], rn[:, 0:48], tempc[:, 0:1])
        # bounce rq to seg layout [128, 3]
        nc.sync.dma_start(t["rq_flat"][b].ap().rearrange("(a c) -> a c", a=NBH), rqf[:])
        rq_seg = sp.tile([128, NSEG], f32, tag="rqseg", name=f"rqseg_{b}")
        nc.sync.dma_start(rq_seg[:],
                          t["rq_flat"][b].ap().rearrange("(s p) -> p s", s=NSEG))
        # rk broadcast [128, 144] via sel matmuls (fp32)
        psrk = ps_pool.tile([128, NSEG * 48], f32, tag="dw1", name="rkps")
        for s in range(NSEG):
            nc.tensor.matmul(psrk[:, 48 * s:48 * s + 48],
                             sel3[:, 128 * s:128 * s + 128], rn[:, 48:96],
                             start=True, stop=True)
        rk_bc = sp.tile([128, NSEG * 48], f32, tag="rkbc", name=f"rkbc_{b}")
        nc.vector.tensor_copy(rk_bc[:], psrk[:])
        # G_seg loads directly from the compact flat qk buffer
        G_seg = sp.tile([128, NSEG * 48], f32, tag="gseg", name=f"gseg_{b}")
        nc.sync.dma_start(
            G_seg[:].rearrange("p (s d) -> p s d", s=NSEG),
            t["g3c_all"][b].ap()[0:NQK].rearrange("(s p d) -> p s d", s=NSEG, p=128))

        A = sp.tile([128, NSEG * 48], f32, tag="A", name=f"A_{b}")
        seg = lambda tl, s: tl[:, 48 * s:48 * s + 48]
        for s in range(NSEG):
            nc.vector.scalar_tensor_tensor(
                out=seg(A, s), in0=seg(G_seg, s), scalar=rq_seg[:, s:s + 1],
                in1=seg(rk_bc, s), op0=OP.mult, op1=OP.mult)

        m1 = sp.tile([128, 8 * NSEG], f32, tag="m1", name=f"m1_{b}")
        m2 = sp.tile([128, 8 * NSEG], f32, tag="m2", name=f"m2_{b}")
        m3 = sp.tile([128, 8 * NSEG], f32, tag="m3", name=f"m3_{b}")
        At1 = sp.tile([128, NSEG * 48], f32, tag="At1", name=f"At1_{b}")
        At2 = sp.tile([128, NSEG * 48], f32, tag="At2", name=f"At2_{b}")
        for s in range(NSEG):
            nc.vector.max(m1[:, 8 * s:8 * s + 8], seg(A, s))
            nc.vector.match_replace(seg(At1, s), m1[:, 8 * s:8 * s + 8], seg(A, s), -1e30)
            nc.vector.max(m2[:, 8 * s:8 * s + 8], seg(At1, s))
            nc.vector.match_replace(seg(At2, s), m2[:, 8 * s:8 * s + 8], seg(At1, s), -1e30)
            nc.vector.max(m3[:, 8 * s:8 * s + 8], seg(At2, s))

        rowst = sp.tile([128, NSEG], f32, tag="rowst", name=f"rowst_{b}")   # -rowmax
        nc.vector.reduce_max(rowst[:], m1[:].rearrange("p (s e) -> p s e", e=8), axis=AX.X)
        nc.vector.tensor_scalar_mul(rowst[:], rowst[:], -1.0)
        t24 = sp.tile([128, NSEG], f32, tag="t24", name=f"t24_{b}")
        nc.vector.tensor_reduce(t24[:], m3[:].rearrange("p (s e) -> p s e", e=8),
                                axis=AX.X, op=OP.min)
        t12 = sp.tile([128, NSEG], f32, tag="t12", name=f"t12_{b}")
        m2v = m2[:].rearrange("p (s e) -> p s e", e=8)
        nc.vector.tensor_copy(t12[:], m2v[:, :, 3])

        e1 = sp.tile([128, NSEG * 48], f32, tag="e1", name=f"e1_{b}")
        p1 = sp.tile([128, NSEG * 48], f32, tag="p1", name=f"p1_{b}")
        Z1 = sp.tile([128, NSEG], f32, tag="Z1", name=f"Z1_{b}")
        for s in range(NSEG):
            nc.scalar.activation(seg(e1, s), seg(A, s), AF.Exp,
                                 bias=rowst[:, s:s + 1], scale=1.0)
            nc.vector.scalar_tensor_tensor(
                out=seg(p1, s), in0=seg(A, s), scalar=t24[:, s:s + 1],
                in1=seg(e1, s), op0=OP.is_ge, op1=OP.mult,
                accum_out=Z1[:, s:s + 1])
        r1 = sp.tile([128, NSEG], f32, tag="r1", name=f"r1_{b}")
        nc.vector.reciprocal(r1[:], Z1[:])
        e2 = sp.tile([128, NSEG * 48], f32, tag="e2", name=f"e2_{b}")
        p2 = sp.tile([128, NSEG * 48], f32, tag="p2", name=f"p2_{b}")
        Z2 = sp.tile([128, NSEG], f32, tag="Z2", name=f"Z2_{b}")
        for s in range(NSEG):
            nc.scalar.activation(seg(e2, s), seg(p1, s), AF.Exp,
                                 bias=0.0, scale=r1[:, s:s + 1])
            nc.vector.scalar_tensor_tensor(
                out=seg(p2, s), in0=seg(A, s), scalar=t12[:, s:s + 1],
                in1=seg(e2, s), op0=OP.is_ge, op1=OP.mult,
                accum_out=Z2[:, s:s + 1])
        r2 = sp.tile([128, NSEG], f32, tag="r2", name=f"r2_{b}")
        nc.vector.reciprocal(r2[:], Z2[:])
        r1p = sp.tile([128, NSEG], f32, tag="r1p", name=f"r1p_{b}")
        nc.vector.tensor_scalar_mul(r1p[:], r1[:], attns_bc[:, 0:1])
        r2p = sp.tile([128, NSEG], f32, tag="r2p", name=f"r2p_{b}")
        nc.vector.tensor_scalar_mul(r2p[:], r2[:], attns_bc[:, 1:2])

        ac = sp.tile([128, NSEG * 48], f32, tag="ac", name=f"ac_{b}")
        tmpc = sp.tile([128, NSEG * 48], f32, tag="tmpc", name=f"tmpc_{b}")
        for s in range(NSEG):
            nc.vector.tensor_scalar_mul(seg(tmpc, s), seg(p2, s), r2p[:, s:s + 1])
            nc.vector.scalar_tensor_tensor(
                out=seg(ac, s), in0=seg(p1, s), scalar=r1p[:, s:s + 1],
                in1=seg(tmpc, s), op0=OP.mult, op1=OP.add)

        # ---- transpose attn_comb per head: bounce + PE transpose + bounce ----
        nc.sync.dma_start(
            t["ac_flat"][b].ap().rearrange("(s p d) -> p s d", s=NSEG, p=128),
            ac[:].rearrange("p (s d) -> p s d", s=NSEG))
        acm = sp.tile([48, NBH * 48], f32, tag="acm", name=f"acm_{b}")
        nc.sync.dma_start(
            acm[:].rearrange("c (bh d) -> c bh d", bh=NBH),
            t["ac_flat"][b].ap().rearrange("(bh c d) -> c bh d", bh=NBH, c=48))
        acT_sb = sp.tile([48, NBH * 48], bf16, tag="acTsb", name=f"acTsb_{b}")
        acm3 = acm[:].rearrange("c (bh d) -> c bh d", bh=NBH)
        pst = ps_pool.tile([48, 8 * 48], f32, tag="dw2", name="acTps")
        for hh in range(8):
            nc.tensor.matmul(pst[:, 48 * hh:48 * hh + 48], acm3[:, hh, :],
                             ident[0:48, 0:48], is_transpose=True,
                             start=True, stop=True)
        nc.vector.tensor_copy(acT_sb[:], pst[:])
        nc.sync.dma_start(
            t["acT_dram"][b].ap().rearrange("(bh d c) -> d bh c", bh=NBH, d=48),
            acT_sb[:].rearrange("d (bh c) -> d bh c", bh=NBH))

    # =============== phase C: attn@v, proj, prompt, blend ===============
    def heads_mv(mv):
        return {h for h in range(HEADS)
                if max(0, 128 * mv - 48 * h) < min(48, 128 * mv + 128 - 48 * h)}

    def phase_C(b):
        # gk loads (chr on partitions 0:64, detg on 64:128) + alpha first
        # (independent of attention)
        gkbig = cp.tile([128, NL], bf16, tag="c_gk", name=f"gk_{b}")
        nc.sync.dma_start(gkbig[0:64, :], t["gk0_s"][b])
        nc.sync.dma_start(gkbig[64:128, :], t["gk1_s"][b])
        al16 = cp.tile([128, NL], bf16, tag="c_al", name=f"al16_{b}")
        for ck in range(NCHUNK):
            psal = ps_pool.tile([128, CH], f32, tag=f"dw{ck % 4}", name="alps")
            nc.tensor.matmul(psal[:], zrep16[64:128, :],
                             gkbig[64:128, ck * CH:(ck + 1) * CH],
                             start=True, stop=True, tile_position=(64, 0))
            nc.scalar.copy(al16[:, ck * CH:(ck + 1) * CH], psal[:])

        atb = [cp.tile([128, DIM], bf16, tag=f"c_atb{mv}", name=f"atb{mv}_{b}") for mv in range(3)]
        acT3 = t["acT_dram"][b].ap().rearrange("(bh d c) -> bh d c", bh=NBH, d=48)
        for mv in range(3):
            nc.vector.memset(atb[mv][:], 0.0)
        for h in range(HEADS):
            for mv in range(3):
                d0 = max(0, 128 * mv - 48 * h)
                d1 = min(48, 128 * mv + 128 - 48 * h)
                if d0 >= d1:
                    continue
                p0 = 48 * h + d0 - 128 * mv
                nc.sync.dma_start(atb[mv][p0:p0 + (d1 - d0), 48 * h:48 * h + 48],
                                  acT3[h, d0:d1, :])

        oattn = [cp.tile([128, NL], bf16, tag=f"c_big{mo}", name=f"oattn{mo}_{b}") for mo in range(3)]
        for mo in range(3):
            mvs = [mv for mv in range(3) if heads_mv(mv) & heads_mv(mo)]
            for ck in range(NCHUNK):
                pso = ps_pool.tile([128, CH], f32, tag=f"dw{ck % 4}", name="avps")
                for ii, mv in enumerate(mvs):
                    nc.tensor.matmul(pso[:], atb[mv][:, 128 * mo:128 * mo + 128],
                                     vcm[b][mv][:, ck * CH:(ck + 1) * CH],
                                     start=(ii == 0), stop=(ii == len(mvs) - 1))
                nc.vector.tensor_copy(oattn[mo][:, ck * CH:(ck + 1) * CH], pso[:])

        out0 = [cp.tile([128, NL], bf16, tag=f"c_out{mo}", name=f"out0{mo}_{b}") for mo in range(3)]
        for mo in range(3):
            for ck in range(NCHUNK):
                psp = ps_pool.tile([128, CH], f32, tag=f"dw{ck % 4}", name="projps")
                for k in range(3):
                    nc.tensor.matmul(psp[:], wprojT[k][:, 128 * mo:128 * mo + 128],
                                     oattn[k][:, ck * CH:(ck + 1) * CH],
                                     start=(k == 0), stop=(k == 2))
                ecopy(out0[mo][:, ck * CH:(ck + 1) * CH], psp[:])

        # prompt branches: g16 -> gate (broadcast-stationary matmul, sigmoid)
        # -> gated in place over the gk half
        gate_bc = cp.tile([128, NL], bf16, tag="c_gate", name=f"gate_{b}")
        for bi, br in enumerate(("chr", "detg")):
            pbase = 64 * bi
            g16 = [cp.tile([128, NL], bf16, tag=f"c_big{mo}", name=f"g16{br}{mo}_{b}") for mo in range(3)]
            for mo in range(3):
                for ck in range(NCHUNK):
                    psg = ps_pool.tile([128, CH], f32, tag=f"dw{ck % 4}", name="gps")
                    for k in range(3):
                        nc.tensor.matmul(psg[:], w1T[br][k][:, 128 * mo:128 * mo + 128],
                                         out0[k][:, ck * CH:(ck + 1) * CH],
                                         start=(k == 0), stop=(k == 2))
                    nc.scalar.activation(g16[mo][:, ck * CH:(ck + 1) * CH], psg[:],
                                         AF.Gelu, bias=b1[br][:, mo:mo + 1], scale=1.0)
            # gate replicated over 64 partitions directly by the matmul
            # (w2rep columns are 64 copies of w2's k-tile)
            for ck in range(NCHUNK):
                psgt = ps_pool.tile([128, CH], f32, tag=f"dw{ck % 4}", name="gateps")
                for k in range(3):
                    nc.tensor.matmul(psgt[pbase:pbase + 64, :], w2r[br][k][:],
                                     g16[k][:, ck * CH:(ck + 1) * CH],
                                     start=(k == 0), stop=(k == 2),
                                     tile_position=(0, pbase))
                nc.scalar.activation(gate_bc[pbase:pbase + 64, ck * CH:(ck + 1) * CH],
                                     psgt[pbase:pbase + 64, :],
                                     AF.Sigmoid, bias=b2col[pbase:pbase + 64, bi:bi + 1],
                                     scale=1.0)
            nc.vector.tensor_mul(gkbig[pbase:pbase + 64, :],
                                 gkbig[pbase:pbase + 64, :],
                                 gate_bc[pbase:pbase + 64, :])

        # wt matmuls: both branches concurrently on disjoint array halves
        prom = {"chr": [], "detg": []}
        for mo in range(3):
            pr_c = cp.tile([128, NL], bf16, tag=f"c_pr{mo}", name=f"prchr{mo}_{b}")
            pr_d = cp.tile([128, NL], bf16, tag=f"c_pr{3 + mo}", name=f"prdetg{mo}_{b}")
            for ck in range(NCHUNK):
                ps_c = ps_pool.tile([128, CH], f32, tag=f"dw{(2 * ck) % 4}", name="wtcps")
                ps_d = ps_pool.tile([128, CH], f32, tag=f"dw{(2 * ck + 1) % 4}", name="wtdps")
                nc.tensor.matmul(ps_c[:], wtTp[0:64, 128 * mo:128 * mo + 128],
                                 gkbig[0:64, ck * CH:(ck + 1) * CH],
                                 start=True, stop=True, tile_position=(0, 0))
                nc.tensor.matmul(ps_d[:], wtTp[64:128, 128 * mo:128 * mo + 128],
                                 gkbig[64:128, ck * CH:(ck + 1) * CH],
                                 start=True, stop=True, tile_position=(64, 0))
                nc.scalar.activation(pr_c[:, ck * CH:(ck + 1) * CH], ps_c[:],
                                     AF.Silu, bias=bt["chr"][:, mo:mo + 1], scale=1.0)
                nc.scalar.activation(pr_d[:, ck * CH:(ck + 1) * CH], ps_d[:],
                                     AF.Silu, bias=bt["detg"][:, mo:mo + 1], scale=1.0)
            prom["chr"].append(pr_c)
            prom["detg"].append(pr_d)

        # blend: out = out0 + chr + al*(detg - chr)  (all bf16, in place on
        # tiles whose values die here)
        for mo in range(3):
            pd = prom["detg"][mo]
            pc = prom["chr"][mo]
            po = out0[mo]
            nc.vector.tensor_sub(pd[:], pd[:], pc[:])      # pd = detg - chr
            nc.vector.tensor_mul(pd[:], pd[:], al16[:])    # pd *= alpha
            nc.gpsimd.tensor_add(po[:], pc[:], po[:])      # po = chr + out0
            nc.vector.tensor_add(pc[:], pd[:], po[:])      # pc = final
            nc.sync.dma_start(t["OUT"][b, mo], pc[:])

    # =============== schedule ===============
    for b in range(B):
        phase_A(b)
        nc.gpsimd.collective_compute(
            "AllReduce", OP.add, replica_groups=[list(range(NCORE))],
            ins=[t["g3c_part"][b].ap().opt()], outs=[t["g3c_all"][b].ap().opt()])
    for b in range(B):
        phase_B(b)
        phase_C(b)


_PROG = None


def _program():
    global _PROG
    if _PROG is None:
        _PROG = build_program()
    return _PROG


def kernel(**inputs):
    from concourse.bass_utils import run_bass_kernel_spmd
    nc = _program()
    consts, _, _ = prep_constants(inputs)
    maps = shard_inputs(inputs, consts)
    res = run_bass_kernel_spmd(nc, maps, list(range(NCORE)))
    out = np.empty((B, DIM, Himg, Wimg), np.float32)
    for ci in range(NCORE):
        o = np.asarray(res.results[ci]["OUT"], dtype=np.float32).reshape(
            B, DIM, ROWS, Wimg)
        out[:, :, ROWS * ci:ROWS * (ci + 1), :] = o
    return out


# revision 24
# speedup vs baseline: 1.2118x; 1.2118x over previous
"""TRN2 Bass kernel for nn_CrossAttention (sparse channel attention + prompt
fusion), sharded spatially over 8 NeuronCores.  Self-contained: builds the
SPMD Bass/Tile program once, shards the full inputs host-side (16 image rows
per core + halo), runs via run_bass_kernel_spmd, and reassembles the output.

Pipelined structure: per-batch Gram AllReduces overlap the other batch's
convolution work and the first batch's attention tail:
  A0 -> AR0 || A1 -> AR1 || B0,C0 -> B1,C1
"""
import sys

for _p in ("/opt/trn_rl_repo", "/root/.axon_site/_ro/trn_rl_repo"):
    if _p not in sys.path:
        sys.path.insert(0, _p)

import numpy as np

B, DIM, HEADS, Himg, Wimg = 2, 384, 8, 128, 128
C = DIM // HEADS            # 48
QKVC = 3 * DIM              # 1152
NCORE = 8
ROWS = Himg // NCORE        # 16 rows per core
NL = ROWS * Wimg            # 2048 local pixels
HR = ROWS + 2               # 18 rows with halo
NH = HR * Wimg              # 2304 halo pixels
PADW = Wimg + 2             # 130
NPAD = HR * PADW            # 2340 padded-free size
NSEG = 3                    # 384 attention rows per batch / 128
NBH = HEADS                 # 8 (per batch)


def build_dw_units():
    """Returns (units, perm) where units is a list of dicts and perm maps
    raw qkv channel -> (in_tile m, in_part p).  44 units per batch.

    unit: raw_base, length, in_tile, in_block(i), in_off, out_kind('q'/'k'/'v'),
          out_tile, out_base (partition base of its out run), j (out col_grp)
    """
    units = []
    for kind, koff in (("q", 0), ("k", DIM)):
        for h in range(HEADS):
            units.append(dict(kind=kind, raw_base=koff + C * h, length=32,
                              out_tile=h // 2, out_base=64 * (h % 2),
                              j=2 * (h % 2), half=False))
            units.append(dict(kind=kind, raw_base=koff + C * h + 32, length=16,
                              out_tile=h // 2, out_base=64 * (h % 2) + 32,
                              j=2 * (h % 2) + 1, half=True))
    for t in range(12):
        units.append(dict(kind="v", raw_base=2 * DIM + 32 * t, length=32,
                          out_tile=t // 4, out_base=32 * (t % 4),
                          j=t % 4, half=False))

    # Three execution sub-passes, each confined to 3 input tiles so only
    # [128, 3*NPAD] of qkv-pad SBUF is live at a time:
    #   sp0: q/k heads 0-3 -> in_tiles 0-2; sp1: q/k heads 4-7 -> 3-5;
    #   sp2: v -> 6-8.
    # Within each sub-pass, each j class has exactly 4 units -> assign
    # distinct i (round robin), giving (i, j) sub-array depth 1 per pass.
    def subpass_of(u):
        if u["kind"] == "v":
            return 2
        return 0 if (u["raw_base"] % DIM) < 4 * C else 1

    jc_ctr = {}
    for u in units:
        key = (subpass_of(u), u["j"])
        o = jc_ctr.get(key, 0)
        jc_ctr[key] = o + 1
        # stagger v's 3-members-per-j classes so all four i values are used
        u["i"] = (u["j"] + o) % 4 if u["kind"] == "v" else o
    assert all(u["i"] < 4 for u in units)

    # pack into input slots within each sub-pass's 3 tiles
    slot_next = {}
    half_open = {}
    for u in units:
        spi, i = subpass_of(u), u["i"]
        key = (spi, i)
        if not u["half"]:
            m = 3 * spi + slot_next.get(key, 0)
            slot_next[key] = slot_next.get(key, 0) + 1
            u["in_tile"], u["in_off"] = m, 0
        else:
            if key in half_open:
                u["in_tile"], u["in_off"] = half_open.pop(key), 16
            else:
                m = 3 * spi + slot_next.get(key, 0)
                slot_next[key] = slot_next.get(key, 0) + 1
                half_open[key] = m
                u["in_tile"], u["in_off"] = m, 0
    assert not half_open, half_open
    assert all(v <= 3 for v in slot_next.values()), slot_next
    for u in units:
        assert 3 * subpass_of(u) <= u["in_tile"] < 3 * subpass_of(u) + 3

    # column base in the dwdiag weight table (per-i column space is shared
    # across the 4 partition groups)
    slot_ctr = {i: 0 for i in range(4)}
    for u in units:
        i = u["i"]
        u["colbase"] = slot_ctr[i] * 9 * 32
        slot_ctr[i] += 1

    # permutation: raw qkv channel -> (m, p)
    perm = np.full((QKVC, 2), -1, np.int64)
    for u in units:
        for r in range(u["length"]):
            raw = u["raw_base"] + r
            p = 32 * u["i"] + u["in_off"] + r
            perm[raw] = (u["in_tile"], p)
    assert (perm >= 0).all()
    return units, perm


def prep_constants(inputs):
    """Build all host-side DRAM input arrays (weights, tables) shared by all cores."""
    import ml_dtypes
    bf16 = ml_dtypes.bfloat16
    units, perm = build_dw_units()
    w_qkv = np.asarray(inputs["w_qkv"], np.float32)      # [1152, 384]
    w_dw = np.asarray(inputs["w_dw"], np.float32).reshape(QKVC, 9)
    w_proj = np.asarray(inputs["w_proj"], np.float32)    # [384, 384]

    # wqkvT: [3 kt, 128, 1152] with cols in PERMUTED order (m*128+p)
    wqkvT = np.zeros((3, 128, QKVC), np.float32)
    for raw in range(QKVC):
        m, p = perm[raw]
        wqkvT[:, :, m * 128 + p] = w_qkv[raw].reshape(3, 128)

    # dwdiag: [128, COLS] bf16; per unit u, tap t a [32,32] block at
    # partitions 32*i..+32, columns colbase(u,t)..+32.
    maxu_per_i = max(sum(1 for u in units if u["i"] == i) for i in range(4))
    COLS = maxu_per_i * 9 * 32
    dwdiag = np.zeros((128, COLS), np.float32)
    for u in units:
        i = u["i"]
        for t in range(9):
            cb = u["colbase"] + t * 32
            for c in range(u["length"]):
                raw = u["raw_base"] + c
                r = u["in_off"] + c
                dwdiag[32 * i + r, cb + c] = w_dw[raw, t]

    def lhsT3(w):   # w [out, in=384] -> [3, 128, out]
        return np.transpose(np.asarray(w, np.float32).reshape(-1, 3, 128), (1, 2, 0)).copy()

    # sel3: [8, 3*128] f32: sel3[j, s*128+p] = 1 if (128s+p)//48 == j
    sel = np.zeros((NBH, NSEG * 128), np.float32)
    for s in range(NSEG):
        for p in range(128):
            rr = 128 * s + p
            sel[rr // 48, s * 128 + p] = 1.0

    # w2rep: [3, 128, 64] — w2's k-tile broadcast across 64 columns, so the
    # gate matmul produces the per-pixel gate replicated over 64 partitions.
    def w2rep(w):
        r = np.asarray(w, np.float32).reshape(3, 128, 1)
        return np.repeat(r, 64, axis=2)

    # wtT_pack: [128, 384] — chr on partitions 0:64, detg on 64:128, for
    # row-packed concurrent branch matmuls.
    wtT_pack = np.concatenate(
        [np.asarray(inputs["chr_wt"], np.float32).T,
         np.asarray(inputs["detg_wt"], np.float32).T], axis=0)

    out = dict(
        wqkvT=wqkvT.astype(bf16),
        dwdiag=dwdiag.astype(bf16),
        wprojT=lhsT3(w_proj).astype(bf16),
        w1T_chr=lhsT3(inputs["chr_w1"]).astype(bf16),
        w1T_detg=lhsT3(inputs["detg_w1"]).astype(bf16),
        w2rep_chr=w2rep(inputs["chr_w2"]).astype(bf16),
        w2rep_detg=w2rep(inputs["detg_w2"]).astype(bf16),
        wtT_pack=wtT_pack.astype(bf16),
        b2col=np.tile(np.asarray([float(np.asarray(inputs["chr_b2"]).ravel()[0]),
                                  float(np.asarray(inputs["detg_b2"]).ravel()[0])],
                                 np.float32), (128, 1)),
        b1_chr=np.ascontiguousarray(np.asarray(inputs["chr_b1"], np.float32).reshape(3, 128).T),
        b1_detg=np.ascontiguousarray(np.asarray(inputs["detg_b1"], np.float32).reshape(3, 128).T),
        bt_chr=np.ascontiguousarray(np.asarray(inputs["chr_bt"], np.float32).reshape(3, 128).T),
        bt_detg=np.ascontiguousarray(np.asarray(inputs["detg_bt"], np.float32).reshape(3, 128).T),
        temp_rep=np.asarray(inputs["temperature"], np.float32).reshape(NBH, 1).copy(),
        attns2=np.asarray(inputs["attns"], np.float32).reshape(1, 2).copy(),
        detg_z=np.asarray(inputs["detg_z"], np.float32).reshape(1, 64).copy(),
        sel3=sel,
    )
    return out, units, perm


def shard_inputs(inputs, consts):
    """Per-core input maps: x slices (bf16, halo-padded), gk slices (bf16)."""
    import ml_dtypes
    bf16 = ml_dtypes.bfloat16
    x = np.asarray(inputs["x"], np.float32)      # [B, 384, 128, 128]
    gk0 = np.asarray(inputs["gk0"], np.float32)  # [B, 64, 128, 128]
    gk1 = np.asarray(inputs["gk1"], np.float32)
    xp = np.pad(x, ((0, 0), (0, 0), (1, 1), (0, 0)))   # zero halo rows
    maps = []
    for ci in range(NCORE):
        r0 = ROWS * ci
        xs = xp[:, :, r0:r0 + HR, :]                       # [B, 384, 18, 128]
        xs = xs.reshape(B, 3, 128, NH)                     # channel-tiled
        g0 = gk0[:, :, r0:r0 + ROWS, :].reshape(B, 64, NL)
        g1 = gk1[:, :, r0:r0 + ROWS, :].reshape(B, 64, NL)
        m = {"x_s": np.ascontiguousarray(xs).astype(bf16),
             "gk0_s": np.ascontiguousarray(g0).astype(bf16),
             "gk1_s": np.ascontiguousarray(g1).astype(bf16)}
        m.update({k: v for k, v in consts.items()})
        maps.append(m)
    return maps


from contextlib import ExitStack


import concourse.bass as bass
import concourse.tile as tile
import concourse.mybir as mybir
from concourse import bacc
from concourse.masks import make_identity

f32 = mybir.dt.float32
bf16 = mybir.dt.bfloat16
AX = mybir.AxisListType
OP = mybir.AluOpType
AF = mybir.ActivationFunctionType
CH = 512          # pixel chunk for most matmuls
NCHUNK = NL // CH  # 4


def build_program():
    units, _ = build_dw_units()
    maxu = max(sum(1 for u in units if u["i"] == i) for i in range(4))
    DWCOLS = maxu * 9 * 32

    nc = bacc.Bacc("TRN2", debug=False, num_devices=NCORE,
                   target_bir_lowering=False)

    def din(name, shape, dt=bf16):
        return nc.dram_tensor(name, list(shape), dt, kind="ExternalInput").ap()

    x_s = din("x_s", (B, 3, 128, NH))
    gk0_s = din("gk0_s", (B, 64, NL))
    gk1_s = din("gk1_s", (B, 64, NL))
    wqkvT_d = din("wqkvT", (3, 128, QKVC))
    dwdiag_d = din("dwdiag", (128, DWCOLS))
    wprojT_d = din("wprojT", (3, 128, DIM))
    w1T_d = {"chr": din("w1T_chr", (3, 128, DIM)), "detg": din("w1T_detg", (3, 128, DIM))}
    w2rep_d = {"chr": din("w2rep_chr", (3, 128, 64)), "detg": din("w2rep_detg", (3, 128, 64))}
    wtTp_d = din("wtT_pack", (128, DIM))
    b1_d = {"chr": din("b1_chr", (128, 3), f32), "detg": din("b1_detg", (128, 3), f32)}
    bt_d = {"chr": din("bt_chr", (128, 3), f32), "detg": din("bt_detg", (128, 3), f32)}
    b2col_d = din("b2col", (128, 2), f32)
    temp_d = din("temp_rep", (NBH, 1), f32)
    attns_d = din("attns2", (1, 2), f32)
    zrow_d = din("detg_z", (1, 64), f32)
    sel3_d = din("sel3", (NBH, NSEG * 128), f32)

    OUT = nc.dram_tensor("OUT", [B, 3, 128, NL], bf16, kind="ExternalOutput").ap()

    # internal DRAM (per batch)
    g3_part = [nc.dram_tensor(f"g3_part{b}", [NBH, 96, 96], f32) for b in range(B)]
    # compact collective buffer: qk cross blocks [(h c) d] then norms [h, 96]
    NQK = NBH * 48 * 48
    g3c_part = [nc.dram_tensor(f"g3c_part{b}", [NQK + NBH * 96], f32) for b in range(B)]
    g3c_all = [nc.dram_tensor(f"g3c_all{b}", [NQK + NBH * 96], f32, addr_space="Shared")
               for b in range(B)]
    rq_flat = [nc.dram_tensor(f"rq_flat{b}", [NSEG * 128], f32) for b in range(B)]
    ac_flat = [nc.dram_tensor(f"ac_flat{b}", [NSEG * 128 * 48], f32) for b in range(B)]
    acT_dram = [nc.dram_tensor(f"acT_dram{b}", [NBH * 48 * 48], bf16) for b in range(B)]
    zb_d = nc.dram_tensor("zb", [64], f32)

    with tile.TileContext(nc) as tc, ExitStack() as ctx:
        _body(tc, ctx, units, locals())
    nc.compile()
    return nc


def _body(tc, ctx, units, t):
    nc = tc.nc
    ec = [0]

    def ecopy(out_ap, in_ap):
        if ec[0] % 2 == 0:
            nc.scalar.copy(out_ap, in_ap)
        else:
            nc.vector.tensor_copy(out_ap, in_ap)
        ec[0] += 1

    wp = ctx.enter_context(tc.tile_pool(name="wp", bufs=1))
    pp = ctx.enter_context(tc.tile_pool(name="pp", bufs=1))     # phase-A big
    cp = ctx.enter_context(tc.tile_pool(name="cp", bufs=1))     # phase-C big
    sp = ctx.enter_context(tc.tile_pool(name="sp", bufs=1))     # small scratch
    ps_pool = ctx.enter_context(tc.tile_pool(name="ps", bufs=2, space="PSUM"))

    # ---------------- constants into SBUF ----------------
    wqkvT = [wp.tile([128, QKVC], bf16, tag=f"wqkv{k}", name=f"wqkv{k}") for k in range(3)]
    for k in range(3):
        nc.sync.dma_start(wqkvT[k][:], t["wqkvT_d"][k])
    dwdiag = wp.tile([128, t["dwdiag_d"].shape[1]], bf16, tag="dwdiag", name="dwdiag")
    nc.sync.dma_start(dwdiag[:], t["dwdiag_d"][:])
    wprojT = [wp.tile([128, DIM], bf16, tag=f"wproj{k}", name=f"wproj{k}") for k in range(3)]
    for k in range(3):
        nc.sync.dma_start(wprojT[k][:], t["wprojT_d"][k])
    w1T, w2r, b1, bt = {}, {}, {}, {}
    for br in ("chr", "detg"):
        w1T[br] = [wp.tile([128, DIM], bf16, tag=f"w1{br}{k}", name=f"w1{br}{k}") for k in range(3)]
        for k in range(3):
            nc.sync.dma_start(w1T[br][k][:], t["w1T_d"][br][k])
        w2r[br] = [wp.tile([128, 64], bf16, tag=f"w2{br}{k}", name=f"w2{br}{k}") for k in range(3)]
        for k in range(3):
            nc.sync.dma_start(w2r[br][k][:], t["w2rep_d"][br][k])
        b1[br] = wp.tile([128, 3], f32, tag=f"b1{br}", name=f"b1{br}")
        nc.sync.dma_start(b1[br][:], t["b1_d"][br][:])
        bt[br] = wp.tile([128, 3], f32, tag=f"bt{br}", name=f"bt{br}")
        nc.sync.dma_start(bt[br][:], t["bt_d"][br][:])
    wtTp = wp.tile([128, DIM], bf16, tag="wtTp", name="wtTp")
    nc.sync.dma_start(wtTp[:], t["wtTp_d"][:])
    b2col = wp.tile([128, 2], f32, tag="b2col", name="b2col")
    nc.sync.dma_start(b2col[:], t["b2col_d"][:])
    tempc = wp.tile([NBH, 1], f32, tag="temp", name="temp")
    nc.sync.dma_start(tempc[:], t["temp_d"][:])
    attns_sb = wp.tile([1, 2], f32, tag="attns", name="attns")
    nc.sync.dma_start(attns_sb[:], t["attns_d"][:])
    zrow = wp.tile([1, 64], f32, tag="zrow", name="zrow")
    nc.sync.dma_start(zrow[:], t["zrow_d"][:])
    sel3 = wp.tile([NBH, NSEG * 128], f32, tag="sel3", name="sel3")
    nc.sync.dma_start(sel3[:], t["sel3_d"][:])
    ident = wp.tile([64, 64], f32, tag="ident", name="ident")
    make_identity(nc, ident[:])
    ones1f = wp.tile([1, 128], f32, tag="ones1f", name="ones1f")
    nc.vector.memset(ones1f[:], 1.0)

    # attns broadcast to all partitions via fp32 K=1 matmul
    ps_a = ps_pool.tile([128, 2], f32, tag="dw0", name="attnsps")
    nc.tensor.matmul(ps_a[:], ones1f[:], attns_sb[:], start=True, stop=True)
    attns_bc = wp.tile([128, 2], f32, tag="attnsbc", name="attnsbc")
    nc.vector.tensor_copy(attns_bc[:], ps_a[:])

    # z-bar prep: z / max(||z||, 1e-12), replicated on partitions 64:128
    # (the alpha matmul's rhs gk-detg lives at partitions 64:128)
    zsq = sp.tile([1, 64], f32, tag="zsq", name="zsq")
    nc.scalar.square(zsq[:], zrow[:])
    zss = sp.tile([1, 1], f32, tag="zss", name="zss")
    nc.vector.reduce_sum(zss[:], zsq[:], axis=AX.X)
    nc.scalar.sqrt(zss[:], zss[:])
    nc.vector.tensor_scalar_max(zss[:], zss[:], 1e-12)
    zrs = sp.tile([1, 1], f32, tag="zrs", name="zrs")
    nc.vector.reciprocal(zrs[:], zss[:])
    zn = sp.tile([1, 64], f32, tag="zn", name="zn")
    nc.vector.tensor_scalar_mul(zn[:], zrow[:], zrs[:, 0:1])
    nc.sync.dma_start(t["zb_d"].ap().rearrange("(a b) -> a b", a=1), zn[:])
    zcol = sp.tile([128, 1], f32, tag="zcol", name="zcol")
    nc.sync.dma_start(zcol[64:128, :], t["zb_d"].ap().rearrange("(p a) -> p a", a=1))
    ones128f = wp.tile([128, 128], f32, tag="ones128f", name="ones128f")
    nc.vector.memset(ones128f[:], 1.0)
    zrep = sp.tile([128, 128], f32, tag="zrep", name="zrep")
    nc.vector.tensor_scalar_mul(zrep[64:128, :], ones128f[64:128, :], zcol[64:128, 0:1])
    zrep16 = wp.tile([128, 128], bf16, tag="zrep16", name="zrep16")
    nc.vector.tensor_copy(zrep16[64:128, :], zrep[64:128, :])

    # ---------------- per-batch tensors ----------------
    vcm = [[pp.tile([128, NL], bf16, tag=f"v{b}_{mv}", name=f"v{b}_{mv}") for mv in range(3)]
           for b in range(B)]

    units_by_sp = [
        [u for u in units if u["kind"] in "qk" and u["raw_base"] % DIM < 4 * C],
        [u for u in units if u["kind"] in "qk" and u["raw_base"] % DIM >= 4 * C],
        [u for u in units if u["kind"] == "v"],
    ]

    # =============== phase A: qkv conv + dw conv + Gram ===============
    def conv_subpass(b, spi, x_sb, qpad, kpad):
        us = units_by_sp[spi]
        # qkv conv for this group's 3 input tiles (perm channels 3*spi..)
        qkvpad = pp.tile([128, 3 * NPAD], bf16, tag="bigA", name=f"qkvpad_{b}")
        for mg in range(3):
            m = 3 * spi + mg
            pv = qkvpad[:, mg * NPAD:(mg + 1) * NPAD].rearrange(
                "p (r w) -> p r w", w=PADW)
            nc.vector.memset(pv[:, :, 0:1], 0.0)
            nc.vector.memset(pv[:, :, PADW - 1:PADW], 0.0)
            for nck in range(6):           # 6 x 384-pixel chunks (3 rows)
                psq = ps_pool.tile([128, 384], f32, tag=f"dw{nck % 4}", name="qkvps")
                for k in range(3):
                    nc.tensor.matmul(
                        psq[:], wqkvT[k][:, m * 128:(m + 1) * 128],
                        x_sb[k][:, nck * 384:(nck + 1) * 384],
                        start=(k == 0), stop=(k == 2))
                ecopy(pv[:, 3 * nck:3 * nck + 3, 1:129],
                      psq[:].rearrange("p (r w) -> p r w", w=128))

        # dw conv sub-pass
        outkeys = sorted({(u["kind"], u["out_tile"]) for u in us})
        for ck in range(NCHUNK):
            pso = {ok: ps_pool.tile([128, CH], f32, tag=f"dw{oi}", name=f"dw{ok[0]}{ok[1]}")
                   for oi, ok in enumerate(outkeys)}
            for tap in range(9):
                dy, dx = tap // 3, tap % 3
                for u in us:
                    mg = u["in_tile"] - 3 * spi
                    src = qkvpad[32 * u["i"]:32 * u["i"] + 32,
                                 mg * NPAD:(mg + 1) * NPAD]
                    rhs = src.rearrange("p (r w) -> p r w", w=PADW)[
                        :, 4 * ck + dy: 4 * ck + dy + 4, dx:dx + 128]
                    lhsT = dwdiag[32 * u["i"]:32 * u["i"] + 32,
                                  u["colbase"] + tap * 32: u["colbase"] + tap * 32 + 32]
                    ob = u["out_base"]
                    out = pso[(u["kind"], u["out_tile"])][ob:ob + 32, :]
                    nc.tensor.matmul(out, lhsT, rhs,
                                     start=(tap == 0), stop=(tap == 8),
                                     tile_position=(32 * u["i"], ob),
                                     skip_group_check=True)
            for (kind, ot), ps in pso.items():
                dst = {"q": qpad, "k": kpad, "v": vcm[b]}[kind][ot]
                if kind == "v":
                    ecopy(dst[:, ck * CH:(ck + 1) * CH], ps[:])
                else:
                    # one copy covers both 48-runs (partitions 0:48 and
                    # 64:112); 48:64 carries junk that nothing reads
                    ecopy(dst[0:112, ck * CH:(ck + 1) * CH], ps[0:112, :])

    def gram_half(b, half, qpad, kpad, g3sb):
        s_pm = pp.tile([128, 16 * 384], bf16, tag="spm", name=f"s_pm_{b}_{half}")
        spm3 = s_pm[:].rearrange("p (c blk) -> p c blk", blk=384)
        for hh in range(4):
            h = 4 * half + hh
            for qk, koff in ((qpad, 0), (kpad, 48)):
                src = qk[h // 2][64 * (h % 2):64 * (h % 2) + 48, :]
                nc.sync.dma_start_transpose(
                    spm3[:, :, 96 * hh + koff: 96 * hh + koff + 48], src)
        for hh in range(4):
            h = 4 * half + hh
            # 128-wide stationary triggers the compiler's fast-weight-load;
            # rows 96:128 of the result are junk (next head's columns).
            # hh=3 has no 128-wide window inside the half, so it stays 96.
            wn = 128 if hh < 3 else 96
            psg = ps_pool.tile([128, 96], f32, tag=f"dw{hh % 4}", name="g3ps")
            for ckk in range(16):
                nc.tensor.matmul(psg[0:wn, :], spm3[:, ckk, 96 * hh:96 * hh + wn],
                                 spm3[:, ckk, 96 * hh:96 * hh + 96],
                                 start=(ckk == 0), stop=(ckk == 15))
            nc.vector.tensor_copy(g3sb[:, 96 * h:96 * (h + 1)],
                                  psg[0:96, :])

    def phase_A(b):
        x_sb = [pp.tile([128, NH], bf16, tag=f"x{k}", name=f"x{k}_{b}") for k in range(3)]
        for k in range(3):
            nc.sync.dma_start(x_sb[k][:], t["x_s"][b, k])
        qpad = [pp.tile([128, NL], bf16, tag=f"qk_{i2}", name=f"qpad{i2}_{b}") for i2 in range(4)]
        kpad = [pp.tile([128, NL], bf16, tag=f"qk_{4 + i2}", name=f"kpad{i2}_{b}") for i2 in range(4)]
        g3sb = pp.tile([96, 8 * 96], f32, tag="g3sb", name=f"g3sb_{b}")

        # q/k sub-passes first, each followed by its Gram half, so the
        # AllReduce can start before the v sub-pass runs.
        for spi in (0, 1):
            conv_subpass(b, spi, x_sb, qpad, kpad)
            gram_half(b, spi, qpad, kpad, g3sb)
        nc.sync.dma_start(
            t["g3_part"][b].ap().rearrange("h r c -> r h c"),
            g3sb[:].rearrange("r (h c) -> r h c", c=96))
        # compact extraction (DRAM->DRAM): qk cross block + the two diagonals
        gp = t["g3_part"][b]
        gc = t["g3c_part"][b]
        NQK = NBH * 48 * 48
        nc.sync.dma_start(
            gc.ap()[0:NQK].rearrange("(h c d) -> h c d", h=NBH, c=48),
            gp.ap()[:, 0:48, 48:96])
        with nc.allow_non_contiguous_dma(reason="96-element diag extraction"):
            nc.sync.dma_start(
                gc.ap()[NQK:NQK + NBH * 96].rearrange("(h c) -> h c", h=NBH)[:, 0:48],
                bass.AP(tensor=gp, offset=0, ap=[[96 * 96, NBH], [97, 48]]))
            nc.sync.dma_start(
                gc.ap()[NQK:NQK + NBH * 96].rearrange("(h c) -> h c", h=NBH)[:, 48:96],
                bass.AP(tensor=gp, offset=48 * 96 + 48, ap=[[96 * 96, NBH], [97, 48]]))
        conv_subpass(b, 2, x_sb, qpad, kpad)

    # =============== phase B: attention matrices (per batch) ===============
    def phase_B(b):
        # norms arrive compact: [8, 96] (qq diag | kk diag)
        norm2 = sp.tile([NBH, 96], f32, tag="norm2", name=f"norm2_{b}")
        NQK = NBH * 48 * 48
        nc.sync.dma_start(
            norm2[:],
            t["g3c_all"][b].ap()[NQK:NQK + NBH * 96].rearrange("(h c) -> h c", h=NBH))
        nc.scalar.sqrt(norm2[:], norm2[:])
        nc.vector.tensor_scalar_max(norm2[:], norm2[:], 1e-12)
        rn = sp.tile([NBH, 96], f32, tag="rn", name=f"rn_{b}")
        nc.vector.reciprocal(rn[:], norm2[:])
        rqf = sp.tile([NBH, 48], f32, tag="rqf", name=f"rqf_{b}")
        nc.vector.tensor_scalar_mul(rqf[:], rn[:, 0:48], tempc[:, 0:1])
        # bounce rq to seg layout [128, 3]
        nc.sync.dma_start(t["rq_flat"][b].ap().rearrange("(a c) -> a c", a=NBH), rqf[:])
        rq_seg = sp.tile([128, NSEG], f32, tag="rqseg", name=f"rqseg_{b}")
        nc.sync.dma_start(rq_seg[:],
                          t["rq_flat"][b].ap().rearrange("(s p) -> p s", s=NSEG))
        # rk broadcast [128, 144] via sel matmuls (fp32)
        psrk = ps_pool.tile([128, NSEG * 48], f32, tag="dw1", name="rkps")
        for s in range(NSEG):
            nc.tensor.matmul(psrk[:, 48 * s:48 * s + 48],
                             sel3[:, 128 * s:128 * s + 128], rn[:, 48:96],
                             start=True, stop=True)
        rk_bc = sp.tile([128, NSEG * 48], f32, tag="rkbc", name=f"rkbc_{b}")
        nc.vector.tensor_copy(rk_bc[:], psrk[:])
        # G_seg loads directly from the compact flat qk buffer
        G_seg = sp.tile([128, NSEG * 48], f32, tag="gseg", name=f"gseg_{b}")
        nc.sync.dma_start(
            G_seg[:].rearrange("p (s d) -> p s d", s=NSEG),
            t["g3c_all"][b].ap()[0:NQK].rearrange("(s p d) -> p s d", s=NSEG, p=128))

        A = sp.tile([128, NSEG * 48], f32, tag="A", name=f"A_{b}")
        seg = lambda tl, s: tl[:, 48 * s:48 * s + 48]
        for s in range(NSEG):
            nc.vector.scalar_tensor_tensor(
                out=seg(A, s), in0=seg(G_seg, s), scalar=rq_seg[:, s:s + 1],
                in1=seg(rk_bc, s), op0=OP.mult, op1=OP.mult)

        m1 = sp.tile([128, 8 * NSEG], f32, tag="m1", name=f"m1_{b}")
        m2 = sp.tile([128, 8 * NSEG], f32, tag="m2", name=f"m2_{b}")
        m3 = sp.tile([128, 8 * NSEG], f32, tag="m3", name=f"m3_{b}")
        At1 = sp.tile([128, NSEG * 48], f32, tag="At1", name=f"At1_{b}")
        At2 = sp.tile([128, NSEG * 48], f32, tag="At2", name=f"At2_{b}")
        for s in range(NSEG):
            nc.vector.max(m1[:, 8 * s:8 * s + 8], seg(A, s))
            nc.vector.match_replace(seg(At1, s), m1[:, 8 * s:8 * s + 8], seg(A, s), -1e30)
            nc.vector.max(m2[:, 8 * s:8 * s + 8], seg(At1, s))
            nc.vector.match_replace(seg(At2, s), m2[:, 8 * s:8 * s + 8], seg(At1, s), -1e30)
            nc.vector.max(m3[:, 8 * s:8 * s + 8], seg(At2, s))

        rowst = sp.tile([128, NSEG], f32, tag="rowst", name=f"rowst_{b}")   # -rowmax
        nc.vector.reduce_max(rowst[:], m1[:].rearrange("p (s e) -> p s e", e=8), axis=AX.X)
        nc.vector.tensor_scalar_mul(rowst[:], rowst[:], -1.0)
        t24 = sp.tile([128, NSEG], f32, tag="t24", name=f"t24_{b}")
        nc.vector.tensor_reduce(t24[:], m3[:].rearrange("p (s e) -> p s e", e=8),
                                axis=AX.X, op=OP.min)
        t12 = sp.tile([128, NSEG], f32, tag="t12", name=f"t12_{b}")
        m2v = m2[:].rearrange("p (s e) -> p s e", e=8)
        nc.vector.tensor_copy(t12[:], m2v[:, :, 3])

        e1 = sp.tile([128, NSEG * 48], f32, tag="e1", name=f"e1_{b}")
        p1 = sp.tile([128, NSEG * 48], f32, tag="p1", name=f"p1_{b}")
        Z1 = sp.tile([128, NSEG], f32, tag="Z1", name=f"Z1_{b}")
        for s in range(NSEG):
            nc.scalar.activation(seg(e1, s), seg(A, s), AF.Exp,
                                 bias=rowst[:, s:s + 1], scale=1.0)
            nc.vector.scalar_tensor_tensor(
                out=seg(p1, s), in0=seg(A, s), scalar=t24[:, s:s + 1],
                in1=seg(e1, s), op0=OP.is_ge, op1=OP.mult,
                accum_out=Z1[:, s:s + 1])
        r1 = sp.tile([128, NSEG], f32, tag="r1", name=f"r1_{b}")
        nc.vector.reciprocal(r1[:], Z1[:])
        e2 = sp.tile([128, NSEG * 48], f32, tag="e2", name=f"e2_{b}")
        p2 = sp.tile([128, NSEG * 48], f32, tag="p2", name=f"p2_{b}")
        Z2 = sp.tile([128, NSEG], f32, tag="Z2", name=f"Z2_{b}")
        for s in range(NSEG):
            nc.scalar.activation(seg(e2, s), seg(p1, s), AF.Exp,
                                 bias=0.0, scale=r1[:, s:s + 1])
            nc.vector.scalar_tensor_tensor(
                out=seg(p2, s), in0=seg(A, s), scalar=t12[:, s:s + 1],
                in1=seg(e2, s), op0=OP.is_ge, op1=OP.mult,
                accum_out=Z2[:, s:s + 1])
        r2 = sp.tile([128, NSEG], f32, tag="r2", name=f"r2_{b}")
        nc.vector.reciprocal(r2[:], Z2[:])
        r1p = sp.tile([128, NSEG], f32, tag="r1p", name=f"r1p_{b}")
        nc.vector.tensor_scalar_mul(r1p[:], r1[:], attns_bc[:, 0:1])
        r2p = sp.tile([128, NSEG], f32, tag="r2p", name=f"r2p_{b}")
        nc.vector.tensor_scalar_mul(r2p[:], r2[:], attns_bc[:, 1:2])

        ac = sp.tile([128, NSEG * 48], f32, tag="ac", name=f"ac_{b}")
        tmpc = sp.tile([128, NSEG * 48], f32, tag="tmpc", name=f"tmpc_{b}")
        for s in range(NSEG):
            nc.vector.tensor_scalar_mul(seg(tmpc, s), seg(p2, s), r2p[:, s:s + 1])
            nc.vector.scalar_tensor_tensor(
                out=seg(ac, s), in0=seg(p1, s), scalar=r1p[:, s:s + 1],
                in1=seg(tmpc, s), op0=OP.mult, op1=OP.add)

        # ---- transpose attn_comb per head: bounce + PE transpose + bounce ----
        nc.sync.dma_start(
            t["ac_flat"][b].ap().rearrange("(s p d) -> p s d", s=NSEG, p=128),
            ac[:].rearrange("p (s d) -> p s d", s=NSEG))
        acm = sp.tile([48, NBH * 48], f32, tag="acm", name=f"acm_{b}")
        nc.sync.dma_start(
            acm[:].rearrange("c (bh d) -> c bh d", bh=NBH),
            t["ac_flat"][b].ap().rearrange("(bh c d) -> c bh d", bh=NBH, c=48))
        acT_sb = sp.tile([48, NBH * 48], bf16, tag="acTsb", name=f"acTsb_{b}")
        acm3 = acm[:].rearrange("c (bh d) -> c bh d", bh=NBH)
        pst = ps_pool.tile([48, 8 * 48], f32, tag="dw2", name="acTps")
        for hh in range(8):
            nc.tensor.matmul(pst[:, 48 * hh:48 * hh + 48], acm3[:, hh, :],
                             ident[0:48, 0:48], is_transpose=True,
                             start=True, stop=True)
        nc.vector.tensor_copy(acT_sb[:], pst[:])
        nc.sync.dma_start(
            t["acT_dram"][b].ap().rearrange("(bh d c) -> d bh c", bh=NBH, d=48),
            acT_sb[:].rearrange("d (bh c) -> d bh c", bh=NBH))

    # =============== phase C: attn@v, proj, prompt, blend ===============
    def heads_mv(mv):
        return {h for h in range(HEADS)
                if max(0, 128 * mv - 48 * h) < min(48, 128 * mv + 128 - 48 * h)}

    def phase_C(b):
        # gk loads (chr on partitions 0:64, detg on 64:128) + alpha first
        # (independent of attention)
        gkbig = cp.tile([128, NL], bf16, tag="c_gk", name=f"gk_{b}")
        nc.sync.dma_start(gkbig[0:64, :], t["gk0_s"][b])
        nc.sync.dma_start(gkbig[64:128, :], t["gk1_s"][b])
        al16 = cp.tile([128, NL], bf16, tag="c_al", name=f"al16_{b}")
        for ck in range(NCHUNK):
            psal = ps_pool.tile([128, CH], f32, tag=f"dw{ck % 4}", name="alps")
            nc.tensor.matmul(psal[:], zrep16[64:128, :],
                             gkbig[64:128, ck * CH:(ck + 1) * CH],
                             start=True, stop=True, tile_position=(64, 0))
            nc.vector.tensor_copy(al16[:, ck * CH:(ck + 1) * CH], psal[:])

        atb = [cp.tile([128, DIM], bf16, tag=f"c_atb{mv}", name=f"atb{mv}_{b}") for mv in range(3)]
        acT3 = t["acT_dram"][b].ap().rearrange("(bh d c) -> bh d c", bh=NBH, d=48)
        for mv in range(3):
            nc.vector.memset(atb[mv][:], 0.0)
        for h in range(HEADS):
            for mv in range(3):
                d0 = max(0, 128 * mv - 48 * h)
                d1 = min(48, 128 * mv + 128 - 48 * h)
                if d0 >= d1:
                    continue
                p0 = 48 * h + d0 - 128 * mv
                nc.sync.dma_start(atb[mv][p0:p0 + (d1 - d0), 48 * h:48 * h + 48],
                                  acT3[h, d0:d1, :])

        oattn = [cp.tile([128, NL], bf16, tag=f"c_big{mo}", name=f"oattn{mo}_{b}") for mo in range(3)]
        for mo in range(3):
            mvs = [mv for mv in range(3) if heads_mv(mv) & heads_mv(mo)]
            for ck in range(NCHUNK):
                pso = ps_pool.tile([128, CH], f32, tag=f"dw{ck % 4}", name="avps")
                for ii, mv in enumerate(mvs):
                    nc.tensor.matmul(pso[:], atb[mv][:, 128 * mo:128 * mo + 128],
                                     vcm[b][mv][:, ck * CH:(ck + 1) * CH],
                                     start=(ii == 0), stop=(ii == len(mvs) - 1))
                nc.vector.tensor_copy(oattn[mo][:, ck * CH:(ck + 1) * CH], pso[:])

        out0 = [cp.tile([128, NL], bf16, tag=f"c_out{mo}", name=f"out0{mo}_{b}") for mo in range(3)]
        for mo in range(3):
            for ck in range(NCHUNK):
                psp = ps_pool.tile([128, CH], f32, tag=f"dw{ck % 4}", name="projps")
                for k in range(3):
                    nc.tensor.matmul(psp[:], wprojT[k][:, 128 * mo:128 * mo + 128],
                                     oattn[k][:, ck * CH:(ck + 1) * CH],
                                     start=(k == 0), stop=(k == 2))
                nc.vector.tensor_copy(out0[mo][:, ck * CH:(ck + 1) * CH], psp[:])

        # prompt branches: g16 -> gate (broadcast-stationary matmul, sigmoid)
        # -> gated in place over the gk half
        gate_bc = cp.tile([128, NL], bf16, tag="c_gate", name=f"gate_{b}")
        for bi, br in enumerate(("chr", "detg")):
            pbase = 64 * bi
            g16 = [cp.tile([128, NL], bf16, tag=f"c_big{mo}", name=f"g16{br}{mo}_{b}") for mo in range(3)]
            for mo in range(3):
                for ck in range(NCHUNK):
                    psg = ps_pool.tile([128, CH], f32, tag=f"dw{ck % 4}", name="gps")
                    for k in range(3):
                        nc.tensor.matmul(psg[:], w1T[br][k][:, 128 * mo:128 * mo + 128],
                                         out0[k][:, ck * CH:(ck + 1) * CH],
                                         start=(k == 0), stop=(k == 2))
                    nc.scalar.activation(g16[mo][:, ck * CH:(ck + 1) * CH], psg[:],
                                         AF.Gelu, bias=b1[br][:, mo:mo + 1], scale=1.0)
            # gate replicated over 64 partitions directly by the matmul
            # (w2rep columns are 64 copies of w2's k-tile)
            for ck in range(NCHUNK):
                psgt = ps_pool.tile([128, CH], f32, tag=f"dw{ck % 4}", name="gateps")
                for k in range(3):
                    nc.tensor.matmul(psgt[pbase:pbase + 64, :], w2r[br][k][:],
                                     g16[k][:, ck * CH:(ck + 1) * CH],
                                     start=(k == 0), stop=(k == 2),
                                     tile_position=(0, pbase))
                nc.scalar.activation(gate_bc[pbase:pbase + 64, ck * CH:(ck + 1) * CH],
                                     psgt[pbase:pbase + 64, :],
                                     AF.Sigmoid, bias=b2col[pbase:pbase + 64, bi:bi + 1],
                                     scale=1.0)
            nc.gpsimd.tensor_mul(gkbig[pbase:pbase + 64, :],
                                 gkbig[pbase:pbase + 64, :],
                                 gate_bc[pbase:pbase + 64, :])

        # wt matmuls: both branches concurrently on disjoint array halves
        prom = {"chr": [], "detg": []}
        for mo in range(3):
            pr_c = cp.tile([128, NL], bf16, tag=f"c_pr{mo}", name=f"prchr{mo}_{b}")
            pr_d = cp.tile([128, NL], bf16, tag=f"c_pr{3 + mo}", name=f"prdetg{mo}_{b}")
            for ck in range(NCHUNK):
                ps_c = ps_pool.tile([128, CH], f32, tag=f"dw{(2 * ck) % 4}", name="wtcps")
                ps_d = ps_pool.tile([128, CH], f32, tag=f"dw{(2 * ck + 1) % 4}", name="wtdps")
                nc.tensor.matmul(ps_c[:], wtTp[0:64, 128 * mo:128 * mo + 128],
                                 gkbig[0:64, ck * CH:(ck + 1) * CH],
                                 start=True, stop=True, tile_position=(0, 0))
                nc.tensor.matmul(ps_d[:], wtTp[64:128, 128 * mo:128 * mo + 128],
                                 gkbig[64:128, ck * CH:(ck + 1) * CH],
                                 start=True, stop=True, tile_position=(64, 0))
                nc.scalar.activation(pr_c[:, ck * CH:(ck + 1) * CH], ps_c[:],
                                     AF.Silu, bias=bt["chr"][:, mo:mo + 1], scale=1.0)
                nc.scalar.activation(pr_d[:, ck * CH:(ck + 1) * CH], ps_d[:],
                                     AF.Silu, bias=bt["detg"][:, mo:mo + 1], scale=1.0)
            prom["chr"].append(pr_c)
            prom["detg"].append(pr_d)

        # blend: out = out0 + chr + al*(detg - chr)  (all bf16, in place on
        # tiles whose values die here)
        for mo in range(3):
            pd = prom["detg"][mo]
            pc = prom["chr"][mo]
            po = out0[mo]
            nc.vector.tensor_sub(pd[:], pd[:], pc[:])      # pd = detg - chr
            nc.vector.tensor_mul(pd[:], pd[:], al16[:])    # pd *= alpha
            nc.gpsimd.tensor_add(po[:], pc[:], po[:])      # po = chr + out0
            nc.vector.tensor_add(pc[:], pd[:], po[:])      # pc = final
            nc.sync.dma_start(t["OUT"][b, mo], pc[:])

    # =============== schedule ===============
    for b in range(B):
        phase_A(b)
        nc.gpsimd.collective_compute(
            "AllReduce", OP.add, replica_groups=[list(range(NCORE))],
            ins=[t["g3c_part"][b].ap().opt()], outs=[t["g3c_all"][b].ap().opt()])
    for b in range(B):
        phase_B(b)
        phase_C(b)


_PROG = None


def _program():
    global _PROG
    if _PROG is None:
        _PROG = build_program()
    return _PROG


def kernel(**inputs):
    from concourse.bass_utils import run_bass_kernel_spmd
    nc = _program()
    consts, _, _ = prep_constants(inputs)
    maps = shard_inputs(inputs, consts)
    res = run_bass_kernel_spmd(nc, maps, list(range(NCORE)))
    out = np.empty((B, DIM, Himg, Wimg), np.float32)
    for ci in range(NCORE):
        o = np.asarray(res.results[ci]["OUT"], dtype=np.float32).reshape(
            B, DIM, ROWS, Wimg)
        out[:, :, ROWS * ci:ROWS * (ci + 1), :] = o
    return out
